# revision 3
# baseline (speedup 1.0000x reference)
"""Distributed attention kernel for Trainium2 (8 NeuronCores).

Sharding: B*H = 2*16 = 32 (batch, head) pairs over 8 cores.
Core c handles batch b = c//4 and global heads 4*(c%4) .. 4*(c%4)+3
(i.e. output columns (c%4)*256 : (c%4+1)*256 of the 1024-wide output).

Per-core kernel (compute in bf16, f32 PSUM accumulation):
  - inputs arrive pre-transposed from host: qT/kT/vT [1024, 2048] bf16,
    weight column slices wq/wk/wv [1024, 256] bf16, mask columns msk
    [128, 16] f32 (0/1), identity [128, 128] f32.
  - projections: QWT/KWT in [d, s] layout, VW in [s, d] layout.
  - scores computed transposed: S_T[k, q] so softmax needs no P transpose;
    the two heads of a projection tile are row-packed (64x128 PE tiling)
    so score pairs run concurrently; exp on ScalarE with the additive key
    mask folded into the per-partition activation bias.
  - PV is column-tiled: head0 weights [128,64] -> output partitions 0:64,
    head1 -> 64:128 of one [128, 512] PSUM tile, so the pair streams
    concurrently through disjoint PE column groups at full array width.
  - sum(exp) per query: exp tiles are accumulated per chunk on VectorE
    (bf16 chained adds, 2x mode); per 128-query block a 1-column matmul
    acc_slice^T @ ones gives sumexp in [q-partition] layout.
  - epilogue: TensorE transposes [128,512]->4x[128,128] (O for 2 heads),
    reciprocal + per-partition scale on VectorE, DMA out (t-major layout,
    host reassembles).
"""

import numpy as np

HEADS = 16
DK = 64
DM = 1024
B = 2
S = 2048
HL = 4           # heads per core
NCOL = HL * DK   # 256 projection cols per core
NM = DM // 128   # 8 m-chunks
NKC = S // 128   # 16 k-chunks
NQC = S // 512   # 4 q-chunks
NSUB = 512 // 128
NBLK = 2 * NKC   # 32 score blocks of 512 per (t, qc); 2 blocks per s-tile
NST = NBLK // 2  # 16 s-tiles per (t, qc)

_CACHE = {}


def _build(loop_n=None):
    from contextlib import ExitStack
    import concourse.bass as bass  # noqa: F401
    import concourse.mybir as mybir
    import concourse.bacc as bacc
    import concourse.tile as tile

    f32 = mybir.dt.float32
    bf16 = mybir.dt.bfloat16
    Exp = mybir.ActivationFunctionType.Exp

    nc = bacc.Bacc("TRN2", target_bir_lowering=False, debug=False, num_devices=8)

    qT = nc.dram_tensor("qT", [DM, S], bf16, kind="ExternalInput").ap()
    kT = nc.dram_tensor("kT", [DM, S], bf16, kind="ExternalInput").ap()
    vT = nc.dram_tensor("vT", [DM, S], bf16, kind="ExternalInput").ap()
    # weights arrive host-swizzled to the SBUF layout [128, NM*NCOL]
    wq = nc.dram_tensor("wq", [128, NM * NCOL], bf16, kind="ExternalInput").ap()
    wk = nc.dram_tensor("wk", [128, NM * NCOL], bf16, kind="ExternalInput").ap()
    wv = nc.dram_tensor("wv", [128, NM * NCOL], bf16, kind="ExternalInput").ap()
    msk = nc.dram_tensor("msk", [128, NKC], f32, kind="ExternalInput").ap()
    ident = nc.dram_tensor("ident", [128, 128], f32, kind="ExternalInput").ap()
    # t-major output: rows [t*2048 + q], 128 cols (heads 2t, 2t+1)
    out = nc.dram_tensor("out", [2 * S, 128], f32, kind="ExternalOutput").ap()

    with tile.TileContext(nc) as tc, ExitStack() as ctx:
        const = ctx.enter_context(tc.tile_pool(name="const", bufs=1))
        bigp = ctx.enter_context(tc.tile_pool(name="bigp", bufs=2, space="PSUM"))
        op = ctx.enter_context(tc.tile_pool(name="op", bufs=4, space="PSUM"))
        ep = ctx.enter_context(tc.tile_pool(name="ep", bufs=24))
        accp = ctx.enter_context(tc.tile_pool(name="accp", bufs=3))
        otsp = ctx.enter_context(tc.tile_pool(name="otsp", bufs=4))
        outp = ctx.enter_context(tc.tile_pool(name="outp", bufs=8))
        rcp = ctx.enter_context(tc.tile_pool(name="rcp", bufs=4))

        # ---- persistent SBUF tensors ----
        xq = const.tile([128, NM * S], bf16, tag="xq")
        xk = const.tile([128, NM * S], bf16, tag="xk")
        xv = const.tile([128, NM * S], bf16, tag="xv")
        wq_sb = const.tile([128, NM * NCOL], bf16, tag="wq")
        wk_sb = const.tile([128, NM * NCOL], bf16, tag="wk")
        wv_sb = const.tile([128, NM * NCOL], bf16, tag="wv")
        m_sb = const.tile([128, NKC], f32, tag="m")
        bias_sb = const.tile([128, NKC], f32, tag="bias")
        id_sb = const.tile([128, 128], f32, tag="id")
        ones_bf = const.tile([128, 1], bf16, tag="ones")
        qwt = const.tile([128, 2 * S], bf16, tag="qwt")    # [d(2 heads), s] x2
        kwt = const.tile([128, 2 * S], bf16, tag="kwt")
        vw = const.tile([128, NKC * NCOL], bf16, tag="vw")

        if loop_n:
            # benchmark variant: run the whole body loop_n times on-device
            ctx.enter_context(tc.For_i(0, loop_n, 1))

        # ---- input DMA: weights/consts, then xk/xq interleaved, then xv ----
        nc.sync.dma_start(out=xk[:, 0:S], in_=kT[0:128, :])
        nc.sync.dma_start(out=wk_sb[:, :], in_=wk)
        for m in range(1, NM):
            nc.sync.dma_start(
                out=xk[:, m * S: (m + 1) * S], in_=kT[m * 128: (m + 1) * 128, :]
            )
        nc.sync.dma_start(out=wq_sb[:, :], in_=wq)
        for m in range(NM):
            nc.sync.dma_start(
                out=xq[:, m * S: (m + 1) * S], in_=qT[m * 128: (m + 1) * 128, :]
            )
        for m in range(NM):
            nc.sync.dma_start(
                out=xv[:, m * S: (m + 1) * S], in_=vT[m * 128: (m + 1) * 128, :]
            )
        nc.sync.dma_start(out=wv_sb[:, :], in_=wv)
        nc.sync.dma_start(out=m_sb[:, :], in_=msk)
        nc.sync.dma_start(out=id_sb[:, :], in_=ident)

        # mask -> additive exp bias: (m - 1) * 1e12 (0 for kept keys, -1e12
        # for masked ones); ones column for the sumexp matmuls
        nc.vector.tensor_scalar(
            out=bias_sb[:, :], in0=m_sb[:, :],
            scalar1=1.0, scalar2=1e12,
            op0=mybir.AluOpType.subtract, op1=mybir.AluOpType.mult,
        )
        nc.vector.memset(ones_bf[:, :], 1.0)

        vw_3d = vw[:, :].rearrange("p (k c) -> p k c", k=NKC)

        def proj_qk(w_sb, x_sb, dst, t, qc, pool=None, ptag="big"):
            pool = pool or bigp
            ps = pool.tile([128, 512], f32, tag=ptag, name=f"pqk{t}_{qc}")
            for m in range(NM):
                nc.tensor.matmul(
                    ps[:, :],
                    lhsT=w_sb[:, m * NCOL + t * 128: m * NCOL + t * 128 + 128],
                    rhs=x_sb[:, m * S + qc * 512: m * S + qc * 512 + 512],
                    start=(m == 0),
                    stop=(m == NM - 1),
                )
            nc.vector.tensor_copy(
                dst[:, t * S + qc * 512: t * S + qc * 512 + 512], ps[:, :]
            )

        def proj_qk_mouter(w_sb, x_sb, dst, t):
            # m-outer: consume each x chunk as its DMA lands (4 live tiles)
            ps = [
                bigp.tile([128, 512], f32, tag="big", name=f"pm{t}_{q}")
                for q in (0, 1)
            ] + [
                op.tile([128, 512], f32, tag="o", name=f"pm{t}_{q}")
                for q in (2, 3)
            ]
            for m in range(NM):
                for qc in range(NQC):
                    nc.tensor.matmul(
                        ps[qc][:, :],
                        lhsT=w_sb[:, m * NCOL + t * 128: m * NCOL + t * 128 + 128],
                        rhs=x_sb[:, m * S + qc * 512: m * S + qc * 512 + 512],
                        start=(m == 0),
                        stop=(m == NM - 1),
                    )
                    if m == NM - 1:
                        nc.vector.tensor_copy(
                            dst[:, t * S + qc * 512: t * S + qc * 512 + 512],
                            ps[qc][:, :],
                        )

        def proj_v(kb):
            ps = bigp.tile([128, NCOL], f32, tag="big", name=f"pv{kb}")
            for m in range(NM):
                nc.tensor.matmul(
                    ps[:, :],
                    lhsT=xv[:, m * S + kb * 128: m * S + kb * 128 + 128],
                    rhs=wv_sb[:, m * NCOL: (m + 1) * NCOL],
                    start=(m == 0),
                    stop=(m == NM - 1),
                )
            nc.vector.tensor_copy(vw_3d[:, kb, :], ps[:, :])

        class Chunk:
            """Incremental emitter for one (t, qc) attention chunk."""

            def __init__(self, t, qc):
                self.t, self.qc = t, qc
                self.s_tiles = [None] * NST
                self.e_tiles = [None] * NST
                self.acc = None
                self.o_ps = None
                self.si = 0
                self.pi = 0

            def emit_s(self, n):
                t, qc = self.t, self.qc
                todo = list(range(self.si, min(self.si + n, NBLK)))
                if not todo:
                    return
                self.si = todo[-1] + 1
                for g in range(0, len(todo), 4):
                    blks = todo[g: g + 4]
                    for blk in blks:
                        st = blk // 2
                        if blk % 2 == 0:
                            self.s_tiles[st] = bigp.tile(
                                [128, 1024], f32, tag="big",
                                name=f"sps{t}_{qc}_{st}"
                            )
                    for blk in blks:
                        kc, a = divmod(blk, 2)
                        st, sc = divmod(blk, 2)
                        nc.tensor.matmul(
                            self.s_tiles[st][:, sc * 512: (sc + 1) * 512],
                            lhsT=kwt[
                                64 * a: 64 * a + 64,
                                t * S + kc * 128: t * S + kc * 128 + 128,
                            ],
                            rhs=qwt[
                                64 * a: 64 * a + 64,
                                t * S + qc * 512: t * S + qc * 512 + 512,
                            ],
                            start=True,
                            stop=True,
                            tile_position=(64 * a, 0),
                        )
                    for blk in blks:
                        st, sc = divmod(blk, 2)
                        if sc == 1:
                            self.e_tiles[st] = ep.tile(
                                [128, 1024], bf16, tag="e",
                                name=f"et{t}_{qc}_{st}"
                            )
                            nc.scalar.activation(
                                self.e_tiles[st][:, :],
                                self.s_tiles[st][:, :],
                                Exp,
                                scale=0.125,
                                bias=bias_sb[:, st: st + 1],
                            )
                            # running sum(exp) over k-chunks, bf16 on VectorE
                            if st == 0:
                                self.acc = accp.tile(
                                    [128, 1024], bf16, tag="acc",
                                    name=f"acc{t}_{qc}"
                                )
                                nc.vector.tensor_copy(
                                    self.acc[:, :], self.e_tiles[0][:, :]
                                )
                            else:
                                nc.vector.tensor_add(
                                    self.acc[:, :],
                                    self.acc[:, :],
                                    self.e_tiles[st][:, :],
                                )

            def emit_pv(self, n):
                t, qc = self.t, self.qc
                if self.o_ps is None:
                    self.o_ps = op.tile(
                        [128, 512], f32, tag="o", name=f"ops{t}_{qc}"
                    )
                blks = list(range(self.pi, min(self.pi + n, NBLK)))
                if not blks:
                    return
                self.pi = blks[-1] + 1
                for blk in blks:
                    kc, a = divmod(blk, 2)
                    st, sc = divmod(blk, 2)
                    # col-tiled pair: head a -> output partitions a*64:(a+1)*64
                    nc.tensor.matmul(
                        self.o_ps[a * 64: a * 64 + 64, :],
                        lhsT=vw_3d[:, kc, (2 * t + a) * 64: (2 * t + a) * 64 + 64],
                        rhs=self.e_tiles[st][:, sc * 512: (sc + 1) * 512],
                        start=(kc == 0),
                        stop=(kc == NKC - 1),
                        # two interleaved accumulation groups on disjoint
                        # partition halves of one bank; the static checker is
                        # partition-unaware but pending-zero is per-partition
                        skip_group_check=True,
                    )

        def epilogue(ck):
            # copy out, transpose O, sumexp matmuls, reciprocal, normalize
            t, qc = ck.t, ck.qc
            ots = otsp.tile([128, 512], f32, tag="ots", name=f"ots{t}_{qc}")
            nc.vector.tensor_copy(ots[:, :], ck.o_ps[:, :])
            for sub in range(NSUB):
                tr = op.tile([128, 512], f32, tag="o", name=f"tr{t}_{qc}_{sub}")
                # cols 0:128 = O^T block (q on partitions, [h0 d64 | h1 d64])
                nc.tensor.transpose(
                    tr[:, 0:128],
                    ots[:, sub * 128: sub * 128 + 128],
                    id_sb[:, :],
                )
                # cols 128+a = sumexp for head a: acc_slice^T @ ones
                for a in range(2):
                    nc.tensor.matmul(
                        tr[:, 128 + a: 129 + a],
                        lhsT=ck.acc[:, a * 512 + sub * 128: a * 512 + sub * 128 + 128],
                        rhs=ones_bf[:, :],
                        start=True,
                        stop=True,
                    )
                rc = rcp.tile([128, 2], f32, tag="rc", name=f"rc{t}_{qc}_{sub}")
                nc.vector.reciprocal_approx_fast(out=rc[:, :], in_=tr[:, 128:130])
                o_out = outp.tile([128, 128], f32, tag="out", name=f"oo{t}_{qc}_{sub}")
                for a in range(2):
                    nc.vector.tensor_scalar_mul(
                        o_out[:, a * 64: (a + 1) * 64],
                        tr[:, a * 64: a * 64 + 64],
                        rc[:, a: a + 1],
                    )
                nc.sync.dma_start(
                    out=out[
                        t * S + qc * 512 + sub * 128:
                        t * S + qc * 512 + sub * 128 + 128, :
                    ],
                    in_=o_out[:, :],
                )

        # ---- schedule: warmup K/Q t0 projections woven with chunk0 scores;
        # then rounds of [scores x4, PV x4] so PE stays saturated and
        # ScalarE never starves.
        chunks = [Chunk(t, qc) for t in range(2) for qc in range(NQC)]

        proj_qk_mouter(wk_sb, xk, kwt, 0)
        for qc in range(NQC):
            proj_qk(wk_sb, xk, kwt, 1, qc)   # runs while xq still arriving
        proj_qk_mouter(wq_sb, xq, qwt, 0)
        for r in range(8):
            chunks[0].emit_s(4)
            if r < 4:
                # Q t1 projection fills chunk-0's exp-paced PE idle; borrow
                # the (still empty) o-pool so score tiles keep both s slots
                proj_qk(wq_sb, xq, qwt, 1, r, pool=op, ptag="o")
            elif r >= 5:
                proj_v(r - 5)   # first V tiles as xv lands
        # V projection + chunk1 scores + chunk0 PV
        for i in range(8):
            if i < 6:
                proj_v(2 * i + 3)
                proj_v(2 * i + 4)
            elif i == 6:
                proj_v(15)
            chunks[0].emit_pv(4)
            chunks[1].emit_s(4)
        epilogue(chunks[0])
        # chunk2 scores + chunk1 PV
        for j in range(8):
            chunks[1].emit_pv(4)
            chunks[2].emit_s(4)
        epilogue(chunks[1])
        # steady state: rounds of [next-chunk scores x4, current PV x4];
        # the final chunk's PV weaves into the second-to-last chunk's rounds
        for ci in range(2, 7):
            for i in range(8):
                chunks[ci + 1].emit_s(4)
                chunks[ci].emit_pv(4)
                if ci == 6 and i >= 2:
                    chunks[7].emit_pv(4)
            epilogue(chunks[ci])
        chunks[7].emit_pv(NBLK)
        epilogue(chunks[7])

    nc.compile()
    return nc


def _get_nc():
    if "nc" not in _CACHE:
        _CACHE["nc"] = _build()
    return _CACHE["nc"]


def _shard_inputs(q, k, v, mask, Wq, Wk, Wv):
    import ml_dtypes

    bf16 = ml_dtypes.bfloat16
    q = np.asarray(q, np.float32)
    k = np.asarray(k, np.float32)
    v = np.asarray(v, np.float32)
    mask = np.asarray(mask, np.float32)
    Wq = np.asarray(Wq, np.float32)
    Wk = np.asarray(Wk, np.float32)
    Wv = np.asarray(Wv, np.float32)

    def _swz(w):
        # [1024, 256] -> SBUF layout [128, 8*256] (row p = concat_m W[m*128+p])
        return np.ascontiguousarray(
            w.reshape(NM, 128, NCOL).transpose(1, 0, 2).reshape(128, NM * NCOL)
        ).astype(bf16)

    ident = np.eye(128, dtype=np.float32)
    qTs = [np.ascontiguousarray(q[b].T).astype(bf16) for b in range(B)]
    kTs = [np.ascontiguousarray(k[b].T).astype(bf16) for b in range(B)]
    vTs = [np.ascontiguousarray(v[b].T).astype(bf16) for b in range(B)]
    msks = [
        np.ascontiguousarray(mask[b].reshape(NKC, 128).T).astype(np.float32)
        for b in range(B)
    ]
    in_maps = []
    for c in range(8):
        b, j = c // 4, c % 4
        sl = slice(j * NCOL, (j + 1) * NCOL)
        in_maps.append(
            {
                "qT": qTs[b],
                "kT": kTs[b],
                "vT": vTs[b],
                "wq": _swz(Wq[:, sl]),
                "wk": _swz(Wk[:, sl]),
                "wv": _swz(Wv[:, sl]),
                "msk": msks[b],
                "ident": ident,
            }
        )
    return in_maps


def _assemble(results):
    """results: list of 8 dicts with 'out' [2*S, 128] -> full [B, S, 1024]."""
    outp = np.empty((B, S, HEADS * DK), np.float32)
    for c in range(8):
        b, j = c // 4, c % 4
        o = np.asarray(results[c]["out"]).reshape(2, S, 128)
        outp[b, :, j * NCOL: j * NCOL + 128] = o[0]
        outp[b, :, j * NCOL + 128: j * NCOL + 256] = o[1]
    return outp


def kernel(q, k, v, mask, Wq, Wk, Wv):
    from concourse.bass_utils import run_bass_kernel_spmd

    nc = _get_nc()
    in_maps = _shard_inputs(q, k, v, mask, Wq, Wk, Wv)
    res = run_bass_kernel_spmd(nc, in_maps, core_ids=list(range(8))).results
    return _assemble(res)


# revision 6
# speedup vs baseline: 1.2230x; 1.2230x over previous
"""Distributed attention kernel for Trainium2 (8 NeuronCores).

Sharding: B*H = 2*16 = 32 (batch, head) pairs over 8 cores.
Core c handles batch b = c//4 and global heads 4*(c%4) .. 4*(c%4)+3
(i.e. output columns (c%4)*256 : (c%4+1)*256 of the 1024-wide output).

Per-core kernel (compute in bf16, f32 PSUM accumulation):
  - inputs arrive pre-transposed from host: qT/kT/vT [1024, 2048] bf16,
    weight column slices wq/wk/wv [1024, 256] bf16, mask columns msk
    [128, 16] f32 (0/1), identity [128, 128] f32.
  - projections: QWT/KWT in [d, s] layout, VW in [s, d] layout.
  - scores computed transposed: S_T[k, q] so softmax needs no P transpose;
    the two heads of a projection tile are row-packed (64x128 PE tiling)
    so score pairs run concurrently; exp on ScalarE with the additive key
    mask folded into the per-partition activation bias.
  - PV is column-tiled: head0 weights [128,64] -> output partitions 0:64,
    head1 -> 64:128 of one [128, 512] PSUM tile, so the pair streams
    concurrently through disjoint PE column groups at full array width.
  - sum(exp) per query: exp tiles are accumulated per chunk on VectorE
    (bf16 chained adds, 2x mode); per 128-query block a 1-column matmul
    acc_slice^T @ ones gives sumexp in [q-partition] layout.
  - epilogue: TensorE transposes [128,512]->4x[128,128] (O for 2 heads),
    reciprocal + per-partition scale on VectorE, DMA out (t-major layout,
    host reassembles).
"""

import numpy as np

HEADS = 16
DK = 64
DM = 1024
B = 2
S = 2048
HL = 4           # heads per core
NCOL = HL * DK   # 256 projection cols per core
NM = DM // 128   # 8 m-chunks
NKC = S // 128   # 16 k-chunks
NQC = S // 512   # 4 q-chunks
NSUB = 512 // 128
NBLK = 2 * NKC   # 32 score blocks of 512 per (t, qc); 2 blocks per s-tile
NST = NBLK // 2  # 16 s-tiles per (t, qc)

_CACHE = {}


def _build(loop_n=None):
    from contextlib import ExitStack
    import concourse.bass as bass  # noqa: F401
    import concourse.mybir as mybir
    import concourse.bacc as bacc
    import concourse.tile as tile

    f32 = mybir.dt.float32
    bf16 = mybir.dt.bfloat16
    Exp = mybir.ActivationFunctionType.Exp

    nc = bacc.Bacc("TRN2", target_bir_lowering=False, debug=False, num_devices=8)

    qT = nc.dram_tensor("qT", [DM, S], bf16, kind="ExternalInput").ap()
    kT = nc.dram_tensor("kT", [DM, S], bf16, kind="ExternalInput").ap()
    vT = nc.dram_tensor("vT", [DM, S], bf16, kind="ExternalInput").ap()
    # weights arrive host-swizzled to the SBUF layout [128, NM*NCOL]
    wq = nc.dram_tensor("wq", [128, NM * NCOL], bf16, kind="ExternalInput").ap()
    wk = nc.dram_tensor("wk", [128, NM * NCOL], bf16, kind="ExternalInput").ap()
    wv = nc.dram_tensor("wv", [128, NM * NCOL], bf16, kind="ExternalInput").ap()
    msk = nc.dram_tensor("msk", [128, NKC], f32, kind="ExternalInput").ap()
    ident = nc.dram_tensor("ident", [128, 128], f32, kind="ExternalInput").ap()
    # t-major output: rows [t*2048 + q], 128 cols (heads 2t, 2t+1)
    out = nc.dram_tensor("out", [2 * S, 128], f32, kind="ExternalOutput").ap()

    with tile.TileContext(nc) as tc, ExitStack() as ctx:
        const = ctx.enter_context(tc.tile_pool(name="const", bufs=1))
        bigp = ctx.enter_context(tc.tile_pool(name="bigp", bufs=2, space="PSUM"))
        op = ctx.enter_context(tc.tile_pool(name="op", bufs=4, space="PSUM"))
        ep = ctx.enter_context(tc.tile_pool(name="ep", bufs=24))
        accp = ctx.enter_context(tc.tile_pool(name="accp", bufs=3))
        otsp = ctx.enter_context(tc.tile_pool(name="otsp", bufs=4))
        outp = ctx.enter_context(tc.tile_pool(name="outp", bufs=8))
        rcp = ctx.enter_context(tc.tile_pool(name="rcp", bufs=4))

        # ---- persistent SBUF tensors ----
        xq = const.tile([128, NM * S], bf16, tag="xq")
        xk = const.tile([128, NM * S], bf16, tag="xk")
        xv = const.tile([128, NM * S], bf16, tag="xv")
        wq_sb = const.tile([128, NM * NCOL], bf16, tag="wq")
        wk_sb = const.tile([128, NM * NCOL], bf16, tag="wk")
        wv_sb = const.tile([128, NM * NCOL], bf16, tag="wv")
        m_sb = const.tile([128, NKC], f32, tag="m")
        bias_sb = const.tile([128, NKC], f32, tag="bias")
        id_sb = const.tile([128, 128], f32, tag="id")
        ones_bf = const.tile([128, 1], bf16, tag="ones")
        qwt = const.tile([128, 2 * S], bf16, tag="qwt")    # [d(2 heads), s] x2
        kwt = const.tile([128, 2 * S], bf16, tag="kwt")
        vw = const.tile([128, NKC * NCOL], bf16, tag="vw")

        if loop_n:
            # benchmark variant: run the whole body loop_n times on-device
            ctx.enter_context(tc.For_i(0, loop_n, 1))

        # ---- input DMA: weights first, then xk/xq chunk-interleaved so the
        # combined K+Q t0 projection can start as soon as possible; xv last
        nc.sync.dma_start(out=wk_sb[:, :], in_=wk)
        nc.sync.dma_start(out=wq_sb[:, :], in_=wq)
        for m in range(NM):
            nc.sync.dma_start(
                out=xk[:, m * S: (m + 1) * S], in_=kT[m * 128: (m + 1) * 128, :]
            )
            nc.sync.dma_start(
                out=xq[:, m * S: (m + 1) * S], in_=qT[m * 128: (m + 1) * 128, :]
            )
        for m in range(NM):
            nc.sync.dma_start(
                out=xv[:, m * S: (m + 1) * S], in_=vT[m * 128: (m + 1) * 128, :]
            )
        nc.sync.dma_start(out=wv_sb[:, :], in_=wv)
        nc.sync.dma_start(out=m_sb[:, :], in_=msk)
        nc.sync.dma_start(out=id_sb[:, :], in_=ident)

        # mask -> additive exp bias: (m - 1) * 1e12 (0 for kept keys, -1e12
        # for masked ones); ones column for the sumexp matmuls
        nc.vector.tensor_scalar(
            out=bias_sb[:, :], in0=m_sb[:, :],
            scalar1=1.0, scalar2=1e12,
            op0=mybir.AluOpType.subtract, op1=mybir.AluOpType.mult,
        )
        nc.vector.memset(ones_bf[:, :], 1.0)

        vw_3d = vw[:, :].rearrange("p (k c) -> p k c", k=NKC)

        def proj_qk(w_sb, x_sb, dst, t, qc, pool=None, ptag="big"):
            pool = pool or bigp
            ps = pool.tile([128, 512], f32, tag=ptag, name=f"pqk{t}_{qc}")
            for m in range(NM):
                nc.tensor.matmul(
                    ps[:, :],
                    lhsT=w_sb[:, m * NCOL + t * 128: m * NCOL + t * 128 + 128],
                    rhs=x_sb[:, m * S + qc * 512: m * S + qc * 512 + 512],
                    start=(m == 0),
                    stop=(m == NM - 1),
                )
            nc.vector.tensor_copy(
                dst[:, t * S + qc * 512: t * S + qc * 512 + 512], ps[:, :]
            )

        def proj_kq_t0():
            # combined K+Q t=0 projection, m-outer so each (xk_m, xq_m) DMA
            # chunk is consumed as it lands; 8 psum regions = 2 bigp slots
            # split in bank halves + 4 op slots
            big = [bigp.tile([128, 1024], f32, tag="big", name=f"pw{q}")
                   for q in (0, 1)]
            kps = [big[0][:, 0:512], big[0][:, 512:1024],
                   big[1][:, 0:512], big[1][:, 512:1024]]
            qps = [op.tile([128, 512], f32, tag="o", name=f"pwq{q}")[:, :]
                   for q in range(4)]
            for m in range(NM):
                for qc in range(NQC):
                    for w_sb, x_sb, dst, ps in (
                        (wk_sb, xk, kwt, kps[qc]),
                        (wq_sb, xq, qwt, qps[qc]),
                    ):
                        nc.tensor.matmul(
                            ps,
                            lhsT=w_sb[:, m * NCOL: m * NCOL + 128],
                            rhs=x_sb[:, m * S + qc * 512: m * S + qc * 512 + 512],
                            start=(m == 0),
                            stop=(m == NM - 1),
                        )
                        if m == NM - 1:
                            nc.vector.tensor_copy(
                                dst[:, qc * 512: qc * 512 + 512], ps
                            )

        def proj_v(kb):
            ps = op.tile([128, NCOL], f32, tag="o", name=f"pv{kb}")
            for m in range(NM):
                nc.tensor.matmul(
                    ps[:, :],
                    lhsT=xv[:, m * S + kb * 128: m * S + kb * 128 + 128],
                    rhs=wv_sb[:, m * NCOL: (m + 1) * NCOL],
                    start=(m == 0),
                    stop=(m == NM - 1),
                )
            nc.vector.tensor_copy(vw_3d[:, kb, :], ps[:, :])

        class Chunk:
            """Incremental emitter for one (t, qc) attention chunk."""

            def __init__(self, t, qc):
                self.t, self.qc = t, qc
                self.s_tiles = [None] * NST
                self.e_tiles = [None] * NST
                self.acc = None
                self.o_ps = None
                self.si = 0
                self.pi = 0

            def emit_s(self, n):
                t, qc = self.t, self.qc
                todo = list(range(self.si, min(self.si + n, NBLK)))
                if not todo:
                    return
                self.si = todo[-1] + 1
                for g in range(0, len(todo), 4):
                    blks = todo[g: g + 4]
                    for blk in blks:
                        st = blk // 2
                        if blk % 2 == 0:
                            self.s_tiles[st] = bigp.tile(
                                [128, 1024], f32, tag="big",
                                name=f"sps{t}_{qc}_{st}"
                            )
                    for blk in blks:
                        kc, a = divmod(blk, 2)
                        st, sc = divmod(blk, 2)
                        nc.tensor.matmul(
                            self.s_tiles[st][:, sc * 512: (sc + 1) * 512],
                            lhsT=kwt[
                                64 * a: 64 * a + 64,
                                t * S + kc * 128: t * S + kc * 128 + 128,
                            ],
                            rhs=qwt[
                                64 * a: 64 * a + 64,
                                t * S + qc * 512: t * S + qc * 512 + 512,
                            ],
                            start=True,
                            stop=True,
                            tile_position=(64 * a, 0),
                        )
                    for blk in blks:
                        st, sc = divmod(blk, 2)
                        if sc == 1:
                            self.e_tiles[st] = ep.tile(
                                [128, 1024], bf16, tag="e",
                                name=f"et{t}_{qc}_{st}"
                            )
                            nc.scalar.activation(
                                self.e_tiles[st][:, :],
                                self.s_tiles[st][:, :],
                                Exp,
                                scale=0.125,
                                bias=bias_sb[:, st: st + 1],
                            )
                            # running sum(exp) over k-chunks, bf16 on VectorE
                            if st == 0:
                                self.acc = accp.tile(
                                    [128, 1024], bf16, tag="acc",
                                    name=f"acc{t}_{qc}"
                                )
                                nc.vector.tensor_copy(
                                    self.acc[:, :], self.e_tiles[0][:, :]
                                )
                            else:
                                nc.vector.tensor_add(
                                    self.acc[:, :],
                                    self.acc[:, :],
                                    self.e_tiles[st][:, :],
                                )

            def emit_pv(self, n):
                t, qc = self.t, self.qc
                if self.o_ps is None:
                    self.o_ps = op.tile(
                        [128, 512], f32, tag="o", name=f"ops{t}_{qc}"
                    )
                blks = list(range(self.pi, min(self.pi + n, NBLK)))
                if not blks:
                    return
                self.pi = blks[-1] + 1
                for blk in blks:
                    kc, a = divmod(blk, 2)
                    st, sc = divmod(blk, 2)
                    # col-tiled pair: head a -> output partitions a*64:(a+1)*64
                    nc.tensor.matmul(
                        self.o_ps[a * 64: a * 64 + 64, :],
                        lhsT=vw_3d[:, kc, (2 * t + a) * 64: (2 * t + a) * 64 + 64],
                        rhs=self.e_tiles[st][:, sc * 512: (sc + 1) * 512],
                        start=(kc == 0),
                        stop=(kc == NKC - 1),
                        # two interleaved accumulation groups on disjoint
                        # partition halves of one bank; the static checker is
                        # partition-unaware but pending-zero is per-partition
                        skip_group_check=True,
                    )

        def epilogue(ck):
            # copy out, transpose O, sumexp matmuls, reciprocal, normalize
            t, qc = ck.t, ck.qc
            ots = otsp.tile([128, 512], f32, tag="ots", name=f"ots{t}_{qc}")
            nc.vector.tensor_copy(ots[:, :], ck.o_ps[:, :])
            for sub in range(NSUB):
                tr = op.tile([128, 512], f32, tag="o", name=f"tr{t}_{qc}_{sub}")
                # cols 0:128 = O^T block (q on partitions, [h0 d64 | h1 d64])
                nc.tensor.transpose(
                    tr[:, 0:128],
                    ots[:, sub * 128: sub * 128 + 128],
                    id_sb[:, :],
                )
                # cols 128+a = sumexp for head a: acc_slice^T @ ones
                for a in range(2):
                    nc.tensor.matmul(
                        tr[:, 128 + a: 129 + a],
                        lhsT=ck.acc[:, a * 512 + sub * 128: a * 512 + sub * 128 + 128],
                        rhs=ones_bf[:, :],
                        start=True,
                        stop=True,
                    )
                rc = rcp.tile([128, 2], f32, tag="rc", name=f"rc{t}_{qc}_{sub}")
                nc.vector.reciprocal_approx_fast(out=rc[:, :], in_=tr[:, 128:130])
                o_out = outp.tile([128, 128], f32, tag="out", name=f"oo{t}_{qc}_{sub}")
                for a in range(2):
                    nc.vector.tensor_scalar_mul(
                        o_out[:, a * 64: (a + 1) * 64],
                        tr[:, a * 64: a * 64 + 64],
                        rc[:, a: a + 1],
                    )
                nc.sync.dma_start(
                    out=out[
                        t * S + qc * 512 + sub * 128:
                        t * S + qc * 512 + sub * 128 + 128, :
                    ],
                    in_=o_out[:, :],
                )

        # ---- schedule: combined K+Q t0 warmup (DMA-paced), then chunk0
        # scores woven with t1 projections; V projection leads chunk0 PV by
        # one round; chunk1 scores start early so ScalarE never starves.
        chunks = [Chunk(t, qc) for t in range(2) for qc in range(NQC)]

        proj_kq_t0()
        for r in range(8):
            chunks[0].emit_s(4)
            if r < 4:
                proj_qk(wk_sb, xk, kwt, 1, r, pool=op, ptag="o")
            else:
                proj_qk(wq_sb, xq, qwt, 1, r - 4, pool=op, ptag="o")
                chunks[1].emit_s(2)
            if r == 7:
                proj_v(0)
                proj_v(1)
        # V projection + chunk1 scores + chunk0 PV
        for i in range(8):
            if i < 7:
                proj_v(2 * i + 2)
                proj_v(2 * i + 3)
            chunks[0].emit_pv(4)
            chunks[1].emit_s(3)
        epilogue(chunks[0])
        # chunk2 scores + chunk1 PV
        for j in range(8):
            chunks[1].emit_pv(4)
            chunks[2].emit_s(4)
        epilogue(chunks[1])
        # steady state: rounds of [next-chunk scores x4, current PV x4];
        # the final chunk's PV weaves into the second-to-last chunk's rounds
        for ci in range(2, 7):
            for i in range(8):
                chunks[ci + 1].emit_s(4)
                chunks[ci].emit_pv(4)
                if ci == 6 and i >= 2:
                    chunks[7].emit_pv(4)
            epilogue(chunks[ci])
        chunks[7].emit_pv(NBLK)
        epilogue(chunks[7])

    nc.compile()
    return nc


def _get_nc():
    if "nc" not in _CACHE:
        _CACHE["nc"] = _build()
    return _CACHE["nc"]


def _shard_inputs(q, k, v, mask, Wq, Wk, Wv):
    import ml_dtypes

    bf16 = ml_dtypes.bfloat16
    q = np.asarray(q, np.float32)
    k = np.asarray(k, np.float32)
    v = np.asarray(v, np.float32)
    mask = np.asarray(mask, np.float32)
    Wq = np.asarray(Wq, np.float32)
    Wk = np.asarray(Wk, np.float32)
    Wv = np.asarray(Wv, np.float32)

    def _swz(w):
        # [1024, 256] -> SBUF layout [128, 8*256] (row p = concat_m W[m*128+p])
        return np.ascontiguousarray(
            w.reshape(NM, 128, NCOL).transpose(1, 0, 2).reshape(128, NM * NCOL)
        ).astype(bf16)

    ident = np.eye(128, dtype=np.float32)
    qTs = [np.ascontiguousarray(q[b].T).astype(bf16) for b in range(B)]
    kTs = [np.ascontiguousarray(k[b].T).astype(bf16) for b in range(B)]
    vTs = [np.ascontiguousarray(v[b].T).astype(bf16) for b in range(B)]
    msks = [
        np.ascontiguousarray(mask[b].reshape(NKC, 128).T).astype(np.float32)
        for b in range(B)
    ]
    in_maps = []
    for c in range(8):
        b, j = c // 4, c % 4
        sl = slice(j * NCOL, (j + 1) * NCOL)
        in_maps.append(
            {
                "qT": qTs[b],
                "kT": kTs[b],
                "vT": vTs[b],
                "wq": _swz(Wq[:, sl]),
                "wk": _swz(Wk[:, sl]),
                "wv": _swz(Wv[:, sl]),
                "msk": msks[b],
                "ident": ident,
            }
        )
    return in_maps


def _assemble(results):
    """results: list of 8 dicts with 'out' [2*S, 128] -> full [B, S, 1024]."""
    outp = np.empty((B, S, HEADS * DK), np.float32)
    for c in range(8):
        b, j = c // 4, c % 4
        o = np.asarray(results[c]["out"]).reshape(2, S, 128)
        outp[b, :, j * NCOL: j * NCOL + 128] = o[0]
        outp[b, :, j * NCOL + 128: j * NCOL + 256] = o[1]
    return outp


def kernel(q, k, v, mask, Wq, Wk, Wv):
    from concourse.bass_utils import run_bass_kernel_spmd

    nc = _get_nc()
    in_maps = _shard_inputs(q, k, v, mask, Wq, Wk, Wv)
    res = run_bass_kernel_spmd(nc, in_maps, core_ids=list(range(8))).results
    return _assemble(res)


# revision 8
# speedup vs baseline: 1.2251x; 1.0017x over previous
"""Distributed attention kernel for Trainium2 (8 NeuronCores).

Sharding: B*H = 2*16 = 32 (batch, head) pairs over 8 cores.
Core c handles batch b = c//4 and global heads 4*(c%4) .. 4*(c%4)+3
(i.e. output columns (c%4)*256 : (c%4+1)*256 of the 1024-wide output).

Per-core kernel (compute in bf16, f32 PSUM accumulation):
  - inputs arrive pre-transposed from host: qT/kT/vT [1024, 2048] bf16,
    weight column slices wq/wk/wv [1024, 256] bf16, mask columns msk
    [128, 16] f32 (0/1), identity [128, 128] f32.
  - projections: QWT/KWT in [d, s] layout, VW in [s, d] layout.
  - scores computed transposed: S_T[k, q] so softmax needs no P transpose;
    the two heads of a projection tile are row-packed (64x128 PE tiling)
    so score pairs run concurrently; exp on ScalarE with the additive key
    mask folded into the per-partition activation bias.
  - PV is column-tiled: head0 weights [128,64] -> output partitions 0:64,
    head1 -> 64:128 of one [128, 512] PSUM tile, so the pair streams
    concurrently through disjoint PE column groups at full array width.
  - sum(exp) per query: exp tiles are accumulated per chunk on VectorE
    (bf16 chained adds, 2x mode); per 128-query block a 1-column matmul
    acc_slice^T @ ones gives sumexp in [q-partition] layout.
  - epilogue: TensorE transposes [128,512]->4x[128,128] (O for 2 heads),
    reciprocal + per-partition scale on VectorE, DMA out (t-major layout,
    host reassembles).
"""

import numpy as np

HEADS = 16
DK = 64
DM = 1024
B = 2
S = 2048
HL = 4           # heads per core
NCOL = HL * DK   # 256 projection cols per core
NM = DM // 128   # 8 m-chunks
NKC = S // 128   # 16 k-chunks
NQC = S // 512   # 4 q-chunks
NSUB = 512 // 128
NBLK = 2 * NKC   # 32 score blocks of 512 per (t, qc); 2 blocks per s-tile
NST = NBLK // 2  # 16 s-tiles per (t, qc)

_CACHE = {}


def _build(loop_n=None):
    from contextlib import ExitStack
    import concourse.bass as bass  # noqa: F401
    import concourse.mybir as mybir
    import concourse.bacc as bacc
    import concourse.tile as tile

    f32 = mybir.dt.float32
    bf16 = mybir.dt.bfloat16
    Exp = mybir.ActivationFunctionType.Exp

    nc = bacc.Bacc("TRN2", target_bir_lowering=False, debug=False, num_devices=8)

    qT = nc.dram_tensor("qT", [DM, S], bf16, kind="ExternalInput").ap()
    kT = nc.dram_tensor("kT", [DM, S], bf16, kind="ExternalInput").ap()
    vT = nc.dram_tensor("vT", [DM, S], bf16, kind="ExternalInput").ap()
    # weights arrive host-swizzled to the SBUF layout [128, NM*NCOL]
    wq = nc.dram_tensor("wq", [128, NM * NCOL], bf16, kind="ExternalInput").ap()
    wk = nc.dram_tensor("wk", [128, NM * NCOL], bf16, kind="ExternalInput").ap()
    wv = nc.dram_tensor("wv", [128, NM * NCOL], bf16, kind="ExternalInput").ap()
    msk = nc.dram_tensor("msk", [128, NKC], f32, kind="ExternalInput").ap()
    ident = nc.dram_tensor("ident", [128, 128], f32, kind="ExternalInput").ap()
    # t-major output: rows [t*2048 + q], 128 cols (heads 2t, 2t+1)
    out = nc.dram_tensor("out", [2 * S, 128], f32, kind="ExternalOutput").ap()

    with tile.TileContext(nc) as tc, ExitStack() as ctx:
        const = ctx.enter_context(tc.tile_pool(name="const", bufs=1))
        bigp = ctx.enter_context(tc.tile_pool(name="bigp", bufs=2, space="PSUM"))
        op = ctx.enter_context(tc.tile_pool(name="op", bufs=4, space="PSUM"))
        ep = ctx.enter_context(tc.tile_pool(name="ep", bufs=24))
        accp = ctx.enter_context(tc.tile_pool(name="accp", bufs=3))
        otsp = ctx.enter_context(tc.tile_pool(name="otsp", bufs=4))
        outp = ctx.enter_context(tc.tile_pool(name="outp", bufs=8))
        rcp = ctx.enter_context(tc.tile_pool(name="rcp", bufs=4))

        # ---- persistent SBUF tensors ----
        xq = const.tile([128, NM * S], bf16, tag="xq")
        xk = const.tile([128, NM * S], bf16, tag="xk")
        xv = const.tile([128, NM * S], bf16, tag="xv")
        wq_sb = const.tile([128, NM * NCOL], bf16, tag="wq")
        wk_sb = const.tile([128, NM * NCOL], bf16, tag="wk")
        wv_sb = const.tile([128, NM * NCOL], bf16, tag="wv")
        m_sb = const.tile([128, NKC], f32, tag="m")
        bias_sb = const.tile([128, NKC], f32, tag="bias")
        id_sb = const.tile([128, 128], f32, tag="id")
        ones_bf = const.tile([128, 1], bf16, tag="ones")
        qwt = const.tile([128, 2 * S], bf16, tag="qwt")    # [d(2 heads), s] x2
        kwt = const.tile([128, 2 * S], bf16, tag="kwt")
        vw = const.tile([128, NKC * NCOL], bf16, tag="vw")

        if loop_n:
            # benchmark variant: run the whole body loop_n times on-device
            ctx.enter_context(tc.For_i(0, loop_n, 1))

        # ---- input DMA: weights first, then xk/xq chunk-interleaved so the
        # combined K+Q t0 projection can start as soon as possible; xv last
        nc.sync.dma_start(out=wk_sb[:, :], in_=wk)
        nc.sync.dma_start(out=wq_sb[:, :], in_=wq)
        nc.sync.dma_start(out=m_sb[:, :], in_=msk)
        nc.sync.dma_start(out=id_sb[:, :], in_=ident)
        for m in range(NM):
            nc.sync.dma_start(
                out=xk[:, m * S: (m + 1) * S], in_=kT[m * 128: (m + 1) * 128, :]
            )
            nc.sync.dma_start(
                out=xq[:, m * S: (m + 1) * S], in_=qT[m * 128: (m + 1) * 128, :]
            )
        for m in range(NM):
            nc.sync.dma_start(
                out=xv[:, m * S: (m + 1) * S], in_=vT[m * 128: (m + 1) * 128, :]
            )
        nc.sync.dma_start(out=wv_sb[:, :], in_=wv)

        # mask -> additive exp bias: (m - 1) * 1e12 (0 for kept keys, -1e12
        # for masked ones); ones column for the sumexp matmuls
        nc.vector.tensor_scalar(
            out=bias_sb[:, :], in0=m_sb[:, :],
            scalar1=1.0, scalar2=1e12,
            op0=mybir.AluOpType.subtract, op1=mybir.AluOpType.mult,
        )
        nc.vector.memset(ones_bf[:, :], 1.0)

        vw_3d = vw[:, :].rearrange("p (k c) -> p k c", k=NKC)

        def proj_qk(w_sb, x_sb, dst, t, qc, pool=None, ptag="big"):
            pool = pool or bigp
            ps = pool.tile([128, 512], f32, tag=ptag, name=f"pqk{t}_{qc}")
            for m in range(NM):
                nc.tensor.matmul(
                    ps[:, :],
                    lhsT=w_sb[:, m * NCOL + t * 128: m * NCOL + t * 128 + 128],
                    rhs=x_sb[:, m * S + qc * 512: m * S + qc * 512 + 512],
                    start=(m == 0),
                    stop=(m == NM - 1),
                )
            nc.vector.tensor_copy(
                dst[:, t * S + qc * 512: t * S + qc * 512 + 512], ps[:, :]
            )

        def proj_kq_t0():
            # combined K+Q t=0 projection, m-outer so each (xk_m, xq_m) DMA
            # chunk is consumed as it lands; 8 psum regions = 2 bigp slots
            # split in bank halves + 4 op slots
            big = [bigp.tile([128, 1024], f32, tag="big", name=f"pw{q}")
                   for q in (0, 1)]
            kps = [big[0][:, 0:512], big[0][:, 512:1024],
                   big[1][:, 0:512], big[1][:, 512:1024]]
            qps = [op.tile([128, 512], f32, tag="o", name=f"pwq{q}")[:, :]
                   for q in range(4)]
            for m in range(NM):
                for qc in range(NQC):
                    for w_sb, x_sb, dst, ps in (
                        (wk_sb, xk, kwt, kps[qc]),
                        (wq_sb, xq, qwt, qps[qc]),
                    ):
                        nc.tensor.matmul(
                            ps,
                            lhsT=w_sb[:, m * NCOL: m * NCOL + 128],
                            rhs=x_sb[:, m * S + qc * 512: m * S + qc * 512 + 512],
                            start=(m == 0),
                            stop=(m == NM - 1),
                        )
                        if m == NM - 1:
                            nc.vector.tensor_copy(
                                dst[:, qc * 512: qc * 512 + 512], ps
                            )

        def proj_v(kb):
            ps = op.tile([128, NCOL], f32, tag="o", name=f"pv{kb}")
            for m in range(NM):
                nc.tensor.matmul(
                    ps[:, :],
                    lhsT=xv[:, m * S + kb * 128: m * S + kb * 128 + 128],
                    rhs=wv_sb[:, m * NCOL: (m + 1) * NCOL],
                    start=(m == 0),
                    stop=(m == NM - 1),
                )
            nc.vector.tensor_copy(vw_3d[:, kb, :], ps[:, :])

        class Chunk:
            """Incremental emitter for one (t, qc) attention chunk."""

            def __init__(self, t, qc):
                self.t, self.qc = t, qc
                self.s_tiles = [None] * NST
                self.e_tiles = [None] * NST
                self.acc = None
                self.o_ps = None
                self.si = 0
                self.pi = 0

            def emit_s(self, n):
                t, qc = self.t, self.qc
                todo = list(range(self.si, min(self.si + n, NBLK)))
                if not todo:
                    return
                self.si = todo[-1] + 1
                for g in range(0, len(todo), 4):
                    blks = todo[g: g + 4]
                    for blk in blks:
                        st = blk // 2
                        if blk % 2 == 0:
                            self.s_tiles[st] = bigp.tile(
                                [128, 1024], f32, tag="big",
                                name=f"sps{t}_{qc}_{st}"
                            )
                    for blk in blks:
                        kc, a = divmod(blk, 2)
                        st, sc = divmod(blk, 2)
                        nc.tensor.matmul(
                            self.s_tiles[st][:, sc * 512: (sc + 1) * 512],
                            lhsT=kwt[
                                64 * a: 64 * a + 64,
                                t * S + kc * 128: t * S + kc * 128 + 128,
                            ],
                            rhs=qwt[
                                64 * a: 64 * a + 64,
                                t * S + qc * 512: t * S + qc * 512 + 512,
                            ],
                            start=True,
                            stop=True,
                            tile_position=(64 * a, 0),
                        )
                    for blk in blks:
                        st, sc = divmod(blk, 2)
                        if sc == 1:
                            self.e_tiles[st] = ep.tile(
                                [128, 1024], bf16, tag="e",
                                name=f"et{t}_{qc}_{st}"
                            )
                            nc.scalar.activation(
                                self.e_tiles[st][:, :],
                                self.s_tiles[st][:, :],
                                Exp,
                                scale=0.125,
                                bias=bias_sb[:, st: st + 1],
                            )
                            # running sum(exp) over k-chunks, bf16 on VectorE
                            if st == 0:
                                self.acc = accp.tile(
                                    [128, 1024], bf16, tag="acc",
                                    name=f"acc{t}_{qc}"
                                )
                                nc.vector.tensor_copy(
                                    self.acc[:, :], self.e_tiles[0][:, :]
                                )
                            else:
                                nc.vector.tensor_add(
                                    self.acc[:, :],
                                    self.acc[:, :],
                                    self.e_tiles[st][:, :],
                                )

            def emit_pv(self, n):
                t, qc = self.t, self.qc
                if self.o_ps is None:
                    self.o_ps = op.tile(
                        [128, 512], f32, tag="o", name=f"ops{t}_{qc}"
                    )
                blks = list(range(self.pi, min(self.pi + n, NBLK)))
                if not blks:
                    return
                self.pi = blks[-1] + 1
                for blk in blks:
                    kc, a = divmod(blk, 2)
                    st, sc = divmod(blk, 2)
                    # col-tiled pair: head a -> output partitions a*64:(a+1)*64
                    nc.tensor.matmul(
                        self.o_ps[a * 64: a * 64 + 64, :],
                        lhsT=vw_3d[:, kc, (2 * t + a) * 64: (2 * t + a) * 64 + 64],
                        rhs=self.e_tiles[st][:, sc * 512: (sc + 1) * 512],
                        start=(kc == 0),
                        stop=(kc == NKC - 1),
                        # two interleaved accumulation groups on disjoint
                        # partition halves of one bank; the static checker is
                        # partition-unaware but pending-zero is per-partition
                        skip_group_check=True,
                    )

        def epilogue(ck):
            # copy out, transpose O, sumexp matmuls, reciprocal, normalize
            t, qc = ck.t, ck.qc
            ots = otsp.tile([128, 512], f32, tag="ots", name=f"ots{t}_{qc}")
            nc.vector.tensor_copy(ots[:, :], ck.o_ps[:, :])
            for sub in range(NSUB):
                tr = op.tile([128, 512], f32, tag="o", name=f"tr{t}_{qc}_{sub}")
                # cols 0:128 = O^T block (q on partitions, [h0 d64 | h1 d64])
                nc.tensor.transpose(
                    tr[:, 0:128],
                    ots[:, sub * 128: sub * 128 + 128],
                    id_sb[:, :],
                )
                # cols 128+a = sumexp for head a: acc_slice^T @ ones
                for a in range(2):
                    nc.tensor.matmul(
                        tr[:, 128 + a: 129 + a],
                        lhsT=ck.acc[:, a * 512 + sub * 128: a * 512 + sub * 128 + 128],
                        rhs=ones_bf[:, :],
                        start=True,
                        stop=True,
                    )
                rc = rcp.tile([128, 2], f32, tag="rc", name=f"rc{t}_{qc}_{sub}")
                nc.vector.reciprocal_approx_fast(out=rc[:, :], in_=tr[:, 128:130])
                o_out = outp.tile([128, 128], f32, tag="out", name=f"oo{t}_{qc}_{sub}")
                for a in range(2):
                    nc.vector.tensor_scalar_mul(
                        o_out[:, a * 64: (a + 1) * 64],
                        tr[:, a * 64: a * 64 + 64],
                        rc[:, a: a + 1],
                    )
                nc.sync.dma_start(
                    out=out[
                        t * S + qc * 512 + sub * 128:
                        t * S + qc * 512 + sub * 128 + 128, :
                    ],
                    in_=o_out[:, :],
                )

        # ---- schedule: combined K+Q t0 warmup (DMA-paced), then chunk0
        # scores woven with t1 projections; V projection leads chunk0 PV by
        # one round; chunk1 scores start early so ScalarE never starves.
        chunks = [Chunk(t, qc) for t in range(2) for qc in range(NQC)]

        proj_kq_t0()
        for r in range(8):
            chunks[0].emit_s(4)
            if r < 4:
                proj_qk(wk_sb, xk, kwt, 1, r, pool=op, ptag="o")
            else:
                proj_qk(wq_sb, xq, qwt, 1, r - 4, pool=op, ptag="o")
                chunks[1].emit_s(2)
            if r == 7:
                proj_v(0)
                proj_v(1)
        # V projection + chunk1 scores + chunk0 PV
        for i in range(8):
            if i < 7:
                proj_v(2 * i + 2)
                proj_v(2 * i + 3)
            chunks[0].emit_pv(4)
            chunks[1].emit_s(3)
        epilogue(chunks[0])
        # chunk2 scores + chunk1 PV
        for j in range(8):
            chunks[1].emit_pv(4)
            chunks[2].emit_s(4)
        epilogue(chunks[1])
        # steady state: rounds of [next-chunk scores x4, current PV x4];
        # the final chunk's PV weaves into the second-to-last chunk's rounds
        for ci in range(2, 7):
            for i in range(8):
                chunks[ci + 1].emit_s(4)
                chunks[ci].emit_pv(4)
                if ci == 6 and i >= 2:
                    chunks[7].emit_pv(4)
            epilogue(chunks[ci])
        chunks[7].emit_pv(NBLK)
        epilogue(chunks[7])

    nc.compile()
    return nc


def _get_nc():
    if "nc" not in _CACHE:
        _CACHE["nc"] = _build()
    return _CACHE["nc"]


def _shard_inputs(q, k, v, mask, Wq, Wk, Wv):
    import ml_dtypes

    bf16 = ml_dtypes.bfloat16
    q = np.asarray(q, np.float32)
    k = np.asarray(k, np.float32)
    v = np.asarray(v, np.float32)
    mask = np.asarray(mask, np.float32)
    Wq = np.asarray(Wq, np.float32)
    Wk = np.asarray(Wk, np.float32)
    Wv = np.asarray(Wv, np.float32)

    def _swz(w):
        # [1024, 256] -> SBUF layout [128, 8*256] (row p = concat_m W[m*128+p])
        return np.ascontiguousarray(
            w.reshape(NM, 128, NCOL).transpose(1, 0, 2).reshape(128, NM * NCOL)
        ).astype(bf16)

    ident = np.eye(128, dtype=np.float32)
    qTs = [np.ascontiguousarray(q[b].T).astype(bf16) for b in range(B)]
    kTs = [np.ascontiguousarray(k[b].T).astype(bf16) for b in range(B)]
    vTs = [np.ascontiguousarray(v[b].T).astype(bf16) for b in range(B)]
    msks = [
        np.ascontiguousarray(mask[b].reshape(NKC, 128).T).astype(np.float32)
        for b in range(B)
    ]
    in_maps = []
    for c in range(8):
        b, j = c // 4, c % 4
        sl = slice(j * NCOL, (j + 1) * NCOL)
        in_maps.append(
            {
                "qT": qTs[b],
                "kT": kTs[b],
                "vT": vTs[b],
                "wq": _swz(Wq[:, sl]),
                "wk": _swz(Wk[:, sl]),
                "wv": _swz(Wv[:, sl]),
                "msk": msks[b],
                "ident": ident,
            }
        )
    return in_maps


def _assemble(results):
    """results: list of 8 dicts with 'out' [2*S, 128] -> full [B, S, 1024]."""
    outp = np.empty((B, S, HEADS * DK), np.float32)
    for c in range(8):
        b, j = c // 4, c % 4
        o = np.asarray(results[c]["out"]).reshape(2, S, 128)
        outp[b, :, j * NCOL: j * NCOL + 128] = o[0]
        outp[b, :, j * NCOL + 128: j * NCOL + 256] = o[1]
    return outp


def kernel(q, k, v, mask, Wq, Wk, Wv):
    from concourse.bass_utils import run_bass_kernel_spmd

    nc = _get_nc()
    in_maps = _shard_inputs(q, k, v, mask, Wq, Wk, Wv)
    res = run_bass_kernel_spmd(nc, in_maps, core_ids=list(range(8))).results
    return _assemble(res)


# revision 13
# speedup vs baseline: 1.2316x; 1.0053x over previous
"""Distributed attention kernel for Trainium2 (8 NeuronCores).

Sharding: B*H = 2*16 = 32 (batch, head) pairs over 8 cores.
Core c handles batch b = c//4 and global heads 4*(c%4) .. 4*(c%4)+3
(i.e. output columns (c%4)*256 : (c%4+1)*256 of the 1024-wide output).

Per-core kernel (compute in bf16, f32 PSUM accumulation):
  - inputs arrive pre-transposed from host: qT/kT/vT [1024, 2048] bf16,
    weight column slices wq/wk/wv [1024, 256] bf16, mask columns msk
    [128, 16] f32 (0/1), identity [128, 128] f32.
  - projections: QWT/KWT in [d, s] layout, VW in [s, d] layout.
  - scores computed transposed: S_T[k, q] so softmax needs no P transpose;
    the two heads of a projection tile are row-packed (64x128 PE tiling)
    so score pairs run concurrently; exp on ScalarE with the additive key
    mask folded into the per-partition activation bias.
  - PV is column-tiled: head0 weights [128,64] -> output partitions 0:64,
    head1 -> 64:128 of one [128, 512] PSUM tile, so the pair streams
    concurrently through disjoint PE column groups at full array width.
  - sum(exp) per query: exp tiles are accumulated per chunk on VectorE
    (bf16 chained adds, 2x mode); per 128-query block a 1-column matmul
    acc_slice^T @ ones gives sumexp in [q-partition] layout.
  - epilogue: TensorE transposes [128,512]->4x[128,128] (O for 2 heads),
    reciprocal + per-partition scale on VectorE, DMA out (t-major layout,
    host reassembles).
"""

import numpy as np

HEADS = 16
DK = 64
DM = 1024
B = 2
S = 2048
HL = 4           # heads per core
NCOL = HL * DK   # 256 projection cols per core
NM = DM // 128   # 8 m-chunks
NKC = S // 128   # 16 k-chunks
NQC = S // 512   # 4 q-chunks
NSUB = 512 // 128
NBLK = 2 * NKC   # 32 score blocks of 512 per (t, qc); 2 blocks per s-tile
NST = NBLK // 2  # 16 s-tiles per (t, qc)

_CACHE = {}


def _build(loop_n=None):
    from contextlib import ExitStack
    import concourse.bass as bass  # noqa: F401
    import concourse.mybir as mybir
    import concourse.bacc as bacc
    import concourse.tile as tile

    f32 = mybir.dt.float32
    bf16 = mybir.dt.bfloat16
    Exp = mybir.ActivationFunctionType.Exp

    nc = bacc.Bacc("TRN2", target_bir_lowering=False, debug=False, num_devices=8)

    qT = nc.dram_tensor("qT", [DM, S], bf16, kind="ExternalInput").ap()
    kT = nc.dram_tensor("kT", [DM, S], bf16, kind="ExternalInput").ap()
    vT = nc.dram_tensor("vT", [DM, S], bf16, kind="ExternalInput").ap()
    # weights arrive host-swizzled to the SBUF layout [128, NM*NCOL]
    wq = nc.dram_tensor("wq", [128, NM * NCOL], bf16, kind="ExternalInput").ap()
    wk = nc.dram_tensor("wk", [128, NM * NCOL], bf16, kind="ExternalInput").ap()
    wv = nc.dram_tensor("wv", [128, NM * NCOL], bf16, kind="ExternalInput").ap()
    msk = nc.dram_tensor("msk", [128, NKC], f32, kind="ExternalInput").ap()
    ident = nc.dram_tensor("ident", [128, 128], f32, kind="ExternalInput").ap()
    # t-major output: rows [t*2048 + q], 128 cols (heads 2t, 2t+1)
    out = nc.dram_tensor("out", [2 * S, 128], f32, kind="ExternalOutput").ap()

    with tile.TileContext(nc) as tc, ExitStack() as ctx:
        const = ctx.enter_context(tc.tile_pool(name="const", bufs=1))
        bigp = ctx.enter_context(tc.tile_pool(name="bigp", bufs=2, space="PSUM"))
        op = ctx.enter_context(tc.tile_pool(name="op", bufs=4, space="PSUM"))
        ep = ctx.enter_context(tc.tile_pool(name="ep", bufs=24))
        accp = ctx.enter_context(tc.tile_pool(name="accp", bufs=3))
        otsp = ctx.enter_context(tc.tile_pool(name="otsp", bufs=4))
        outp = ctx.enter_context(tc.tile_pool(name="outp", bufs=3))
        rcp = ctx.enter_context(tc.tile_pool(name="rcp", bufs=4))

        # ---- persistent SBUF tensors ----
        xq = const.tile([128, NM * S], bf16, tag="xq")
        xk = const.tile([128, NM * S], bf16, tag="xk")
        xv = const.tile([128, NM * S], bf16, tag="xv")
        wq_sb = const.tile([128, NM * NCOL], bf16, tag="wq")
        wk_sb = const.tile([128, NM * NCOL], bf16, tag="wk")
        wv_sb = const.tile([128, NM * NCOL], bf16, tag="wv")
        m_sb = const.tile([128, NKC], f32, tag="m")
        bias_sb = const.tile([128, NKC], f32, tag="bias")
        id_sb = const.tile([128, 128], f32, tag="id")
        ones_bf = const.tile([128, 1], bf16, tag="ones")
        qwt = const.tile([128, 2 * S], bf16, tag="qwt")    # [d(2 heads), s] x2
        kwt = const.tile([128, 2 * S], bf16, tag="kwt")
        vw = const.tile([128, NKC * NCOL], bf16, tag="vw")

        if loop_n:
            # benchmark variant: run the whole body loop_n times on-device
            ctx.enter_context(tc.For_i(0, loop_n, 1))

        # ---- input DMA: first m-slice of the weights + first x chunks, so
        # the combined K+Q t0 projection starts as early as possible; xv last
        nc.sync.dma_start(out=wk_sb[:, 0:NCOL], in_=wk[:, 0:NCOL])
        nc.sync.dma_start(out=xk[:, 0:S], in_=kT[0:128, :])
        nc.sync.dma_start(out=wq_sb[:, 0:NCOL], in_=wq[:, 0:NCOL])
        nc.sync.dma_start(out=xq[:, 0:S], in_=qT[0:128, :])
        nc.sync.dma_start(out=wk_sb[:, NCOL:], in_=wk[:, NCOL:])
        nc.sync.dma_start(out=wq_sb[:, NCOL:], in_=wq[:, NCOL:])
        nc.sync.dma_start(out=m_sb[:, :], in_=msk)
        nc.sync.dma_start(out=id_sb[:, :], in_=ident)
        for m in range(1, NM):
            nc.sync.dma_start(
                out=xk[:, m * S: (m + 1) * S], in_=kT[m * 128: (m + 1) * 128, :]
            )
            nc.sync.dma_start(
                out=xq[:, m * S: (m + 1) * S], in_=qT[m * 128: (m + 1) * 128, :]
            )
        for m in range(NM):
            nc.sync.dma_start(
                out=xv[:, m * S: (m + 1) * S], in_=vT[m * 128: (m + 1) * 128, :]
            )
        nc.sync.dma_start(out=wv_sb[:, :], in_=wv)

        # mask -> additive exp bias: (m - 1) * 1e12 (0 for kept keys, -1e12
        # for masked ones); ones column for the sumexp matmuls
        nc.vector.tensor_scalar(
            out=bias_sb[:, :], in0=m_sb[:, :],
            scalar1=1.0, scalar2=1e12,
            op0=mybir.AluOpType.subtract, op1=mybir.AluOpType.mult,
        )
        nc.vector.memset(ones_bf[:, :], 1.0)

        vw_3d = vw[:, :].rearrange("p (k c) -> p k c", k=NKC)

        def proj_qk(w_sb, x_sb, dst, t, qc, pool=None, ptag="big"):
            pool = pool or bigp
            ps = pool.tile([128, 512], f32, tag=ptag, name=f"pqk{t}_{qc}")
            for m in range(NM):
                nc.tensor.matmul(
                    ps[:, :],
                    lhsT=w_sb[:, m * NCOL + t * 128: m * NCOL + t * 128 + 128],
                    rhs=x_sb[:, m * S + qc * 512: m * S + qc * 512 + 512],
                    start=(m == 0),
                    stop=(m == NM - 1),
                )
            nc.vector.tensor_copy(
                dst[:, t * S + qc * 512: t * S + qc * 512 + 512], ps[:, :]
            )

        def proj_kq_t0():
            # combined K+Q t=0 projection, m-outer so each (xk_m, xq_m) DMA
            # chunk is consumed as it lands; 8 psum regions = 2 bigp slots
            # split in bank halves + 4 op slots
            big = [bigp.tile([128, 1024], f32, tag="big", name=f"pw{q}")
                   for q in (0, 1)]
            kps = [big[0][:, 0:512], big[0][:, 512:1024],
                   big[1][:, 0:512], big[1][:, 512:1024]]
            qps = [op.tile([128, 512], f32, tag="o", name=f"pwq{q}")[:, :]
                   for q in range(4)]
            for m in range(NM):
                for qc in range(NQC):
                    for w_sb, x_sb, dst, ps in (
                        (wk_sb, xk, kwt, kps[qc]),
                        (wq_sb, xq, qwt, qps[qc]),
                    ):
                        nc.tensor.matmul(
                            ps,
                            lhsT=w_sb[:, m * NCOL: m * NCOL + 128],
                            rhs=x_sb[:, m * S + qc * 512: m * S + qc * 512 + 512],
                            start=(m == 0),
                            stop=(m == NM - 1),
                        )
                        if m == NM - 1:
                            nc.vector.tensor_copy(
                                dst[:, qc * 512: qc * 512 + 512], ps
                            )

        def proj_v(kb):
            ps = op.tile([128, NCOL], f32, tag="o", name=f"pv{kb}")
            for m in range(NM):
                nc.tensor.matmul(
                    ps[:, :],
                    lhsT=xv[:, m * S + kb * 128: m * S + kb * 128 + 128],
                    rhs=wv_sb[:, m * NCOL: (m + 1) * NCOL],
                    start=(m == 0),
                    stop=(m == NM - 1),
                )
            nc.vector.tensor_copy(vw_3d[:, kb, :], ps[:, :])

        class Chunk:
            """Incremental emitter for one (t, qc) attention chunk."""

            def __init__(self, t, qc):
                self.t, self.qc = t, qc
                self.s_tiles = [None] * NST
                self.e_tiles = [None] * NST
                self.acc = None
                self.o_ps = None
                self.si = 0
                self.pi = 0

            def emit_s(self, n):
                t, qc = self.t, self.qc
                todo = list(range(self.si, min(self.si + n, NBLK)))
                if not todo:
                    return
                self.si = todo[-1] + 1
                for g in range(0, len(todo), 4):
                    blks = todo[g: g + 4]
                    for blk in blks:
                        st = blk // 2
                        if blk % 2 == 0:
                            self.s_tiles[st] = bigp.tile(
                                [128, 1024], f32, tag="big",
                                name=f"sps{t}_{qc}_{st}"
                            )
                    for blk in blks:
                        kc, a = divmod(blk, 2)
                        st, sc = divmod(blk, 2)
                        nc.tensor.matmul(
                            self.s_tiles[st][:, sc * 512: (sc + 1) * 512],
                            lhsT=kwt[
                                64 * a: 64 * a + 64,
                                t * S + kc * 128: t * S + kc * 128 + 128,
                            ],
                            rhs=qwt[
                                64 * a: 64 * a + 64,
                                t * S + qc * 512: t * S + qc * 512 + 512,
                            ],
                            start=True,
                            stop=True,
                            tile_position=(64 * a, 0),
                        )
                    for blk in blks:
                        st, sc = divmod(blk, 2)
                        if sc == 1:
                            self.e_tiles[st] = ep.tile(
                                [128, 1024], bf16, tag="e",
                                name=f"et{t}_{qc}_{st}"
                            )
                            nc.scalar.activation(
                                self.e_tiles[st][:, :],
                                self.s_tiles[st][:, :],
                                Exp,
                                scale=0.125,
                                bias=bias_sb[:, st: st + 1],
                            )
                            # running sum(exp) over k-chunks, bf16 on VectorE
                            if st == 0:
                                self.acc = accp.tile(
                                    [128, 1024], bf16, tag="acc",
                                    name=f"acc{t}_{qc}"
                                )
                                nc.vector.tensor_copy(
                                    self.acc[:, :], self.e_tiles[0][:, :]
                                )
                            else:
                                nc.vector.tensor_add(
                                    self.acc[:, :],
                                    self.acc[:, :],
                                    self.e_tiles[st][:, :],
                                )

            def emit_pv(self, n):
                t, qc = self.t, self.qc
                if self.o_ps is None:
                    self.o_ps = op.tile(
                        [128, 512], f32, tag="o", name=f"ops{t}_{qc}"
                    )
                blks = list(range(self.pi, min(self.pi + n, NBLK)))
                if not blks:
                    return
                self.pi = blks[-1] + 1
                for blk in blks:
                    kc, a = divmod(blk, 2)
                    st, sc = divmod(blk, 2)
                    # col-tiled pair: head a -> output partitions a*64:(a+1)*64
                    nc.tensor.matmul(
                        self.o_ps[a * 64: a * 64 + 64, :],
                        lhsT=vw_3d[:, kc, (2 * t + a) * 64: (2 * t + a) * 64 + 64],
                        rhs=self.e_tiles[st][:, sc * 512: (sc + 1) * 512],
                        start=(kc == 0),
                        stop=(kc == NKC - 1),
                        # two interleaved accumulation groups on disjoint
                        # partition halves of one bank; the static checker is
                        # partition-unaware but pending-zero is per-partition
                        skip_group_check=True,
                    )

        def epilogue(ck):
            # copy out, transpose O, sumexp matmuls, reciprocal, normalize
            t, qc = ck.t, ck.qc
            ots = otsp.tile([128, 512], f32, tag="ots", name=f"ots{t}_{qc}")
            nc.vector.tensor_copy(ots[:, :], ck.o_ps[:, :])
            o_out = outp.tile([128, 512], f32, tag="out", name=f"oo{t}_{qc}")
            for sub in range(NSUB):
                tr = op.tile([128, 512], f32, tag="o", name=f"tr{t}_{qc}_{sub}")
                # cols 0:128 = O^T block (q on partitions, [h0 d64 | h1 d64])
                nc.tensor.transpose(
                    tr[:, 0:128],
                    ots[:, sub * 128: sub * 128 + 128],
                    id_sb[:, :],
                )
                # cols 128+a = sumexp for head a: acc_slice^T @ ones
                for a in range(2):
                    nc.tensor.matmul(
                        tr[:, 128 + a: 129 + a],
                        lhsT=ck.acc[:, a * 512 + sub * 128: a * 512 + sub * 128 + 128],
                        rhs=ones_bf[:, :],
                        start=True,
                        stop=True,
                    )
                rc = rcp.tile([128, 2], f32, tag="rc", name=f"rc{t}_{qc}_{sub}")
                nc.vector.reciprocal_approx_fast(out=rc[:, :], in_=tr[:, 128:130])
                for a in range(2):
                    nc.vector.tensor_scalar_mul(
                        o_out[:, sub * 128 + a * 64: sub * 128 + (a + 1) * 64],
                        tr[:, a * 64: a * 64 + 64],
                        rc[:, a: a + 1],
                    )
            # one batched DMA per chunk: SBUF [128, 4, 128] -> DRAM rows
            o_3d = o_out[:, :].rearrange("p (s c) -> p s c", s=NSUB)
            nc.sync.dma_start(
                out=out[t * S + qc * 512: t * S + qc * 512 + 512, :]
                .rearrange("(s p) c -> p s c", s=NSUB),
                in_=o_3d[:, :, :],
            )

        # ---- schedule: combined K+Q t0 warmup (DMA-paced), then chunk0
        # scores woven with t1 projections; V projection leads chunk0 PV by
        # one round; chunk1 scores start early so ScalarE never starves.
        chunks = [Chunk(t, qc) for t in range(2) for qc in range(NQC)]

        proj_kq_t0()
        for r in range(8):
            chunks[0].emit_s(4)
            if r < 4:
                proj_qk(wk_sb, xk, kwt, 1, r, pool=op, ptag="o")
            else:
                proj_qk(wq_sb, xq, qwt, 1, r - 4, pool=op, ptag="o")
                chunks[1].emit_s(2)
            if r == 7:
                proj_v(0)
                proj_v(1)
        # V projection + chunk1 scores + chunk0 PV
        for i in range(8):
            if i < 7:
                proj_v(2 * i + 2)
                proj_v(2 * i + 3)
            chunks[0].emit_pv(4)
            chunks[1].emit_s(3)
        epilogue(chunks[0])
        # chunk2 scores + chunk1 PV
        for j in range(8):
            chunks[1].emit_pv(4)
            chunks[2].emit_s(4)
        epilogue(chunks[1])
        # steady state: rounds of [current PV x4, next-chunk scores x4] (PV
        # first so the PE has ready work while ScalarE catches up on the
        # score->exp slots at chunk boundaries); the final chunk's PV weaves
        # into the second-to-last chunk's rounds
        for ci in range(2, 7):
            for i in range(8):
                chunks[ci].emit_pv(4)
                chunks[ci + 1].emit_s(4)
                if ci == 6 and i >= 2:
                    chunks[7].emit_pv(4)
            epilogue(chunks[ci])
        chunks[7].emit_pv(NBLK)
        epilogue(chunks[7])

    nc.compile()
    return nc


def _get_nc():
    if "nc" not in _CACHE:
        _CACHE["nc"] = _build()
    return _CACHE["nc"]


def _shard_inputs(q, k, v, mask, Wq, Wk, Wv):
    import ml_dtypes

    bf16 = ml_dtypes.bfloat16
    q = np.asarray(q, np.float32)
    k = np.asarray(k, np.float32)
    v = np.asarray(v, np.float32)
    mask = np.asarray(mask, np.float32)
    Wq = np.asarray(Wq, np.float32)
    Wk = np.asarray(Wk, np.float32)
    Wv = np.asarray(Wv, np.float32)

    def _swz(w):
        # [1024, 256] -> SBUF layout [128, 8*256] (row p = concat_m W[m*128+p])
        return np.ascontiguousarray(
            w.reshape(NM, 128, NCOL).transpose(1, 0, 2).reshape(128, NM * NCOL)
        ).astype(bf16)

    ident = np.eye(128, dtype=np.float32)
    qTs = [np.ascontiguousarray(q[b].T).astype(bf16) for b in range(B)]
    kTs = [np.ascontiguousarray(k[b].T).astype(bf16) for b in range(B)]
    vTs = [np.ascontiguousarray(v[b].T).astype(bf16) for b in range(B)]
    msks = [
        np.ascontiguousarray(mask[b].reshape(NKC, 128).T).astype(np.float32)
        for b in range(B)
    ]
    in_maps = []
    for c in range(8):
        b, j = c // 4, c % 4
        sl = slice(j * NCOL, (j + 1) * NCOL)
        in_maps.append(
            {
                "qT": qTs[b],
                "kT": kTs[b],
                "vT": vTs[b],
                "wq": _swz(Wq[:, sl]),
                "wk": _swz(Wk[:, sl]),
                "wv": _swz(Wv[:, sl]),
                "msk": msks[b],
                "ident": ident,
            }
        )
    return in_maps


def _assemble(results):
    """results: list of 8 dicts with 'out' [2*S, 128] -> full [B, S, 1024]."""
    outp = np.empty((B, S, HEADS * DK), np.float32)
    for c in range(8):
        b, j = c // 4, c % 4
        o = np.asarray(results[c]["out"]).reshape(2, S, 128)
        outp[b, :, j * NCOL: j * NCOL + 128] = o[0]
        outp[b, :, j * NCOL + 128: j * NCOL + 256] = o[1]
    return outp


def kernel(q, k, v, mask, Wq, Wk, Wv):
    from concourse.bass_utils import run_bass_kernel_spmd

    nc = _get_nc()
    in_maps = _shard_inputs(q, k, v, mask, Wq, Wk, Wv)
    res = run_bass_kernel_spmd(nc, in_maps, core_ids=list(range(8))).results
    return _assemble(res)
